# revision 8
# baseline (speedup 1.0000x reference)
"""DistanceNetwork kernel for 8 TRN2 NeuronCores.

out[b, i] = dot(query[b], support[b, i]) * rsqrt(max(||support[b,i]||^2, EPS))

Sharding: batch dim (2048) split across 8 cores -> 256 batches/core, no
cross-core communication. Per core, batches sit on SBUF partitions (2 blocks
of 128). For each support row i:
  - VectorE: scalar_tensor_tensor (S*1)*Q with accum_out -> dot column
  - ScalarE: activation(Square) with accum_out         -> sqnorm column
Each engine touches every support element exactly once, so both stay under
the per-core HBM floor (~128 MiB / 358 GB/s ~= 374 us). Epilogue per block:
clip -> sqrt -> reciprocal -> multiply -> DMA out.

Raw bass (no TileContext): this container's walrus can't encode multi-wait
instructions that Tile's scheduler emits, so synchronization is explicit —
standalone wait_ge + then_inc, one semaphore per wait. S-tile loads ride the
SP HWDGE ring (FIFO, back-to-back at full SDMA width); query/output DMAs use
the ACT ring so they don't head-of-line block the S stream.
"""

import sys

if "/opt/trn_rl_repo" not in sys.path:
    sys.path.insert(0, "/opt/trn_rl_repo")

from contextlib import ExitStack

import numpy as np

B, CK, D = 2048, 256, 512
N_CORES = 8
B_SH = B // N_CORES   # 256 batches per core
PB = 128              # batches (partitions) per block
NBLK = B_SH // PB     # 2 blocks
G = 16                # support rows per S-tile DMA (G*D*4 = 32 KiB/partition)
TPB_ = CK // G        # 16 S tiles per block
NTILE = NBLK * TPB_   # 32 S tiles per core
NBUF = 3              # S-tile buffers
EPS = 1e-10

_CACHE = {}


def _build_nc():
    import concourse.bass as bass
    from concourse import mybir

    f32 = mybir.dt.float32
    AF = mybir.ActivationFunctionType
    ALU = mybir.AluOpType

    nc = bass.Bass(
        trn_type="TRN2",
        target_bir_lowering=False,
        debug=False,
        num_devices=N_CORES,
        # the detector rejects benign same-engine in-order WAW on the
        # scratch tiles (engines execute their stream in order + DRAIN);
        # cross-engine hazards here are explicitly semaphored
        detect_race_conditions=False,
    )
    sup = nc.dram_tensor("support", [B_SH, CK, D], f32, kind="ExternalInput")
    qry = nc.dram_tensor("query", [B_SH, D], f32, kind="ExternalInput")
    out = nc.dram_tensor("out", [B_SH, CK], f32, kind="ExternalOutput")

    sup2 = sup.rearrange("b i d -> b (i d)")  # [B_SH, CK*D], contiguous
    qry2 = qry.ap()
    out2 = out.ap()

    with ExitStack() as ctx:
        e = ctx.enter_context
        s_buf = [e(nc.sbuf_tensor(f"s{n}", [PB, G * D], f32)) for n in range(NBUF)]
        q_buf = [e(nc.sbuf_tensor(f"q{b}", [PB, D], f32)) for b in range(NBLK)]
        dot = [e(nc.sbuf_tensor(f"dot{b}", [PB, CK], f32)) for b in range(NBLK)]
        sq = [e(nc.sbuf_tensor(f"sq{b}", [PB, CK], f32)) for b in range(NBLK)]
        scr_v = e(nc.sbuf_tensor("scr_v", [PB, D], f32))
        scr_a = e(nc.sbuf_tensor("scr_a", [PB, D], f32))
        clip = [e(nc.sbuf_tensor(f"clip{b}", [PB, CK], f32)) for b in range(NBLK)]
        mag = [e(nc.sbuf_tensor(f"mag{b}", [PB, CK], f32)) for b in range(NBLK)]
        inv = [e(nc.sbuf_tensor(f"inv{b}", [PB, CK], f32)) for b in range(NBLK)]
        o_t = [e(nc.sbuf_tensor(f"o{b}", [PB, CK], f32)) for b in range(NBLK)]

        # One completion sem per S buffer slot: a slot's DMAs are strictly
        # serialized by the buffer-reuse waits, so per-slot counts are
        # unambiguous (one shared sem would mix +16s of concurrent DMAs).
        sem_s = [e(nc.semaphore(f"sem_s{n}")) for n in range(NBUF)]
        sem_q = [e(nc.semaphore(f"sem_q{b}")) for b in range(NBLK)]
        sem_o = e(nc.semaphore("sem_o"))  # output DMA completions (+16 each)
        sem_v = e(nc.semaphore("sem_v"))  # vector: S tiles fully consumed (+1)
        sem_a = e(nc.semaphore("sem_a"))  # scalar: S tiles fully consumed (+1)
        sem_e = e(nc.semaphore("sem_e"))  # vector epilogue steps (+1 each)
        sem_f = e(nc.semaphore("sem_f"))  # scalar epilogue sqrt done (+1)

        # ---- SP stream: the 32 big S-tile loads, triple buffered ----
        for k in range(NTILE):
            if k >= NBUF:
                # buffer k%NBUF holds tile k-NBUF: both consumers must be done
                nc.sync.wait_ge(sem_v, k - NBUF + 1)
                nc.sync.wait_ge(sem_a, k - NBUF + 1)
            b, j = divmod(k, TPB_)
            row = sup2[b * PB : (b + 1) * PB, j * G * D : (j + 1) * G * D]
            nc.sync.dma_start(out=s_buf[k % NBUF].ap(), in_=row).then_inc(
                sem_s[k % NBUF], 16
            )
        nc.sync.wait_ge(sem_o, 16 * NBLK)  # outputs landed before NEFF exit

        # ---- ACT stream: q loads, square+accumulate, sqrt, output stores ----
        for b in range(NBLK):
            nc.scalar.dma_start(
                out=q_buf[b].ap(), in_=qry2[b * PB : (b + 1) * PB, :]
            ).then_inc(sem_q[b], 16)
        for b in range(NBLK):
            for j in range(TPB_):
                k = b * TPB_ + j
                nc.scalar.wait_ge(sem_s[k % NBUF], 16 * (k // NBUF + 1))
                sb = s_buf[k % NBUF].ap()
                for g in range(G):
                    c = j * G + g
                    ins = nc.scalar.activation(
                        out=scr_a.ap(),
                        in_=sb[:, g * D : (g + 1) * D],
                        func=AF.Square,
                        accum_out=sq[b].ap()[:, c : c + 1],
                    )
                    if g == G - 1:
                        ins.then_inc(sem_a, 1)
            # epilogue: sqrt of clipped norms once vector built clip[b]
            nc.scalar.wait_ge(sem_e, 2 * b + 1)
            nc.scalar.activation(mag[b].ap(), clip[b].ap(), AF.Sqrt).then_inc(
                sem_f, 1
            )
            nc.scalar.wait_ge(sem_e, 2 * b + 2)
            nc.scalar.dma_start(
                out=out2[b * PB : (b + 1) * PB, :], in_=o_t[b].ap()
            ).then_inc(sem_o, 16)

        # ---- DVE stream: fused multiply+reduce (dot), epilogue arithmetic ----
        for b in range(NBLK):
            nc.vector.wait_ge(sem_q[b], 16)
            for j in range(TPB_):
                k = b * TPB_ + j
                nc.vector.wait_ge(sem_s[k % NBUF], 16 * (k // NBUF + 1))
                sb = s_buf[k % NBUF].ap()
                for g in range(G):
                    c = j * G + g
                    ins = nc.vector.scalar_tensor_tensor(
                        out=scr_v.ap(),
                        in0=sb[:, g * D : (g + 1) * D],
                        scalar=1.0,
                        in1=q_buf[b].ap(),
                        op0=ALU.mult,
                        op1=ALU.mult,
                        accum_out=dot[b].ap()[:, c : c + 1],
                    )
                    if g == G - 1:
                        ins.then_inc(sem_v, 1)
            nc.vector.wait_ge(sem_a, TPB_ * (b + 1))
            nc.vector.tensor_scalar_max(clip[b].ap(), sq[b].ap(), EPS).then_inc(
                sem_e, 1
            )
            nc.vector.wait_ge(sem_f, b + 1)
            nc.vector.reciprocal(inv[b].ap(), mag[b].ap())
            nc.vector.tensor_mul(o_t[b].ap(), dot[b].ap(), inv[b].ap()).then_inc(
                sem_e, 1
            )

    return nc


def run(support: np.ndarray, query: np.ndarray, trace: bool = False):
    """Returns (full_output, BassKernelResults)."""
    from concourse.bass_utils import run_bass_kernel_spmd

    if "nc" not in _CACHE:
        _CACHE["nc"] = _build_nc()
    nc = _CACHE["nc"]

    support = np.ascontiguousarray(np.asarray(support, dtype=np.float32))
    query = np.ascontiguousarray(np.asarray(query, dtype=np.float32))
    in_maps = [
        {
            "support": support[c * B_SH : (c + 1) * B_SH],
            "query": query[c * B_SH : (c + 1) * B_SH],
        }
        for c in range(N_CORES)
    ]
    res = run_bass_kernel_spmd(
        nc, in_maps, core_ids=list(range(N_CORES)), trace=trace
    )
    full = np.concatenate([r["out"] for r in res.results], axis=0)
    return full, res


def kernel(support: np.ndarray, query: np.ndarray) -> np.ndarray:
    full, _ = run(support, query, trace=False)
    return full


# revision 9
# speedup vs baseline: 2.3523x; 2.3523x over previous
"""DistanceNetwork kernel for 8 TRN2 NeuronCores.

out[b, i] = dot(query[b], support[b, i]) * rsqrt(max(||support[b,i]||^2, EPS))

Sharding: batch dim (2048) split across 8 cores -> 256 batches/core, no
cross-core communication. Per core, batches sit on SBUF partitions (2 blocks
of 128). For each support row i:
  - VectorE: scalar_tensor_tensor (S*1)*Q with accum_out -> dot column
  - ScalarE: activation(Square) with accum_out         -> sqnorm column
Each engine touches every support element exactly once, so both stay under
the per-core HBM floor (~128 MiB / 358 GB/s ~= 374 us). Epilogue per block:
clip -> sqrt -> reciprocal -> multiply -> DMA out.

Raw bass (no TileContext): this container's walrus can't encode multi-wait
instructions that Tile's scheduler emits, so synchronization is explicit —
standalone wait_ge + then_inc, one semaphore per wait. S-tile loads ride the
SP HWDGE ring (FIFO, back-to-back at full SDMA width); query/output DMAs use
the ACT ring so they don't head-of-line block the S stream.

`repeats` re-emits the whole body N times with cumulative semaphore
thresholds — used by bench.py to measure steady-state per-iteration device
time without host dispatch in the loop.
"""

import sys

if "/opt/trn_rl_repo" not in sys.path:
    sys.path.insert(0, "/opt/trn_rl_repo")

from contextlib import ExitStack

import numpy as np

B, CK, D = 2048, 256, 512
N_CORES = 8
B_SH = B // N_CORES   # 256 batches per core
PB = 128              # batches (partitions) per block
NBLK = B_SH // PB     # 2 blocks
G = 16                # support rows per S-tile DMA (G*D*4 = 32 KiB/partition)
TPB_ = CK // G        # 16 S tiles per block
NTILE = NBLK * TPB_   # 32 S tiles per core per repeat
NBUF = 3              # S-tile buffers
EPS = 1e-10

_CACHE = {}


def _build_nc(repeats: int = 1):
    import concourse.bass as bass
    from concourse import mybir

    f32 = mybir.dt.float32
    AF = mybir.ActivationFunctionType
    ALU = mybir.AluOpType

    nc = bass.Bass(
        trn_type="TRN2",
        target_bir_lowering=False,
        debug=False,
        num_devices=N_CORES,
        # the detector rejects benign same-engine in-order WAW on the
        # scratch tiles (engines execute their stream in order + DRAIN);
        # cross-engine hazards here are explicitly semaphored
        detect_race_conditions=False,
    )
    sup = nc.dram_tensor("support", [B_SH, CK, D], f32, kind="ExternalInput")
    qry = nc.dram_tensor("query", [B_SH, D], f32, kind="ExternalInput")
    out = nc.dram_tensor("out", [B_SH, CK], f32, kind="ExternalOutput")

    sup2 = sup.rearrange("b i d -> b (i d)")  # [B_SH, CK*D], contiguous
    qry2 = qry.ap()
    out2 = out.ap()

    with ExitStack() as ctx:
        e = ctx.enter_context
        s_buf = [e(nc.sbuf_tensor(f"s{n}", [PB, G * D], f32)) for n in range(NBUF)]
        q_buf = [e(nc.sbuf_tensor(f"q{b}", [PB, D], f32)) for b in range(NBLK)]
        dot = [e(nc.sbuf_tensor(f"dot{b}", [PB, CK], f32)) for b in range(NBLK)]
        sq = [e(nc.sbuf_tensor(f"sq{b}", [PB, CK], f32)) for b in range(NBLK)]
        scr_v = e(nc.sbuf_tensor("scr_v", [PB, D], f32))
        scr_a = e(nc.sbuf_tensor("scr_a", [PB, D], f32))
        clip = [e(nc.sbuf_tensor(f"clip{b}", [PB, CK], f32)) for b in range(NBLK)]
        mag = [e(nc.sbuf_tensor(f"mag{b}", [PB, CK], f32)) for b in range(NBLK)]
        inv = [e(nc.sbuf_tensor(f"inv{b}", [PB, CK], f32)) for b in range(NBLK)]
        o_t = [e(nc.sbuf_tensor(f"o{b}", [PB, CK], f32)) for b in range(NBLK)]

        # One completion sem per S buffer slot: a slot's DMAs are strictly
        # serialized by the buffer-reuse waits, so per-slot counts are
        # unambiguous (one shared sem would mix +16s of concurrent DMAs).
        sem_s = [e(nc.semaphore(f"sem_s{n}")) for n in range(NBUF)]
        sem_q = [e(nc.semaphore(f"sem_q{b}")) for b in range(NBLK)]
        sem_o = e(nc.semaphore("sem_o"))  # output DMA completions (+16 each)
        sem_v = e(nc.semaphore("sem_v"))  # vector: S tiles fully consumed (+1)
        sem_a = e(nc.semaphore("sem_a"))  # scalar: S tiles fully consumed (+1)
        sem_e = e(nc.semaphore("sem_e"))  # vector epilogue steps (+1 each)
        sem_f = e(nc.semaphore("sem_f"))  # scalar epilogue sqrt done (+1)

        # ---- SP stream: big S-tile loads, triple buffered ----
        for r in range(repeats):
            for kk in range(NTILE):
                k = r * NTILE + kk
                if k >= NBUF:
                    # slot k%NBUF holds tile k-NBUF: both consumers done?
                    nc.sync.wait_ge(sem_v, k - NBUF + 1)
                    nc.sync.wait_ge(sem_a, k - NBUF + 1)
                b, j = divmod(kk, TPB_)
                row = sup2[b * PB : (b + 1) * PB, j * G * D : (j + 1) * G * D]
                nc.sync.dma_start(out=s_buf[k % NBUF].ap(), in_=row).then_inc(
                    sem_s[k % NBUF], 16
                )
        nc.sync.wait_ge(sem_o, 16 * NBLK * repeats)  # outputs landed

        # ---- ACT stream: q loads, square+accumulate, sqrt, output stores ----
        for r in range(repeats):
            for b in range(NBLK):
                if r > 0:
                    # q_buf[b] may still feed repeat r-1's vector STTs
                    nc.scalar.wait_ge(sem_v, r * NTILE)
                nc.scalar.dma_start(
                    out=q_buf[b].ap(), in_=qry2[b * PB : (b + 1) * PB, :]
                ).then_inc(sem_q[b], 16)
            for b in range(NBLK):
                for j in range(TPB_):
                    k = r * NTILE + b * TPB_ + j
                    nc.scalar.wait_ge(sem_s[k % NBUF], 16 * (k // NBUF + 1))
                    sb = s_buf[k % NBUF].ap()
                    for g in range(G):
                        c = j * G + g
                        ins = nc.scalar.activation(
                            out=scr_a.ap(),
                            in_=sb[:, g * D : (g + 1) * D],
                            func=AF.Square,
                            accum_out=sq[b].ap()[:, c : c + 1],
                        )
                        if g == G - 1:
                            ins.then_inc(sem_a, 1)
                # epilogue: sqrt of clipped norms once vector built clip[b]
                eb = r * NBLK + b
                nc.scalar.wait_ge(sem_e, 2 * eb + 1)
                nc.scalar.activation(mag[b].ap(), clip[b].ap(), AF.Sqrt).then_inc(
                    sem_f, 1
                )
                nc.scalar.wait_ge(sem_e, 2 * eb + 2)
                nc.scalar.dma_start(
                    out=out2[b * PB : (b + 1) * PB, :], in_=o_t[b].ap()
                ).then_inc(sem_o, 16)

        # ---- DVE stream: fused multiply+reduce (dot), epilogue arithmetic ----
        for r in range(repeats):
            for b in range(NBLK):
                nc.vector.wait_ge(sem_q[b], 16 * (r + 1))
                for j in range(TPB_):
                    k = r * NTILE + b * TPB_ + j
                    nc.vector.wait_ge(sem_s[k % NBUF], 16 * (k // NBUF + 1))
                    sb = s_buf[k % NBUF].ap()
                    for g in range(G):
                        c = j * G + g
                        ins = nc.vector.scalar_tensor_tensor(
                            out=scr_v.ap(),
                            in0=sb[:, g * D : (g + 1) * D],
                            scalar=1.0,
                            in1=q_buf[b].ap(),
                            op0=ALU.mult,
                            op1=ALU.mult,
                            accum_out=dot[b].ap()[:, c : c + 1],
                        )
                        if g == G - 1:
                            ins.then_inc(sem_v, 1)
                eb = r * NBLK + b
                nc.vector.wait_ge(sem_a, TPB_ * (NBLK * r + b + 1))
                nc.vector.tensor_scalar_max(clip[b].ap(), sq[b].ap(), EPS).then_inc(
                    sem_e, 1
                )
                nc.vector.wait_ge(sem_f, eb + 1)
                nc.vector.reciprocal(inv[b].ap(), mag[b].ap())
                if r > 0:
                    # o_t[b] may still be read by repeat r-1's output DMA
                    nc.vector.wait_ge(sem_o, 16 * (NBLK * (r - 1) + b + 1))
                nc.vector.tensor_mul(o_t[b].ap(), dot[b].ap(), inv[b].ap()).then_inc(
                    sem_e, 1
                )

    return nc


def run(support: np.ndarray, query: np.ndarray, trace: bool = False):
    """Returns (full_output, BassKernelResults)."""
    from concourse.bass_utils import run_bass_kernel_spmd

    if "nc" not in _CACHE:
        _CACHE["nc"] = _build_nc()
    nc = _CACHE["nc"]

    support = np.ascontiguousarray(np.asarray(support, dtype=np.float32))
    query = np.ascontiguousarray(np.asarray(query, dtype=np.float32))
    in_maps = [
        {
            "support": support[c * B_SH : (c + 1) * B_SH],
            "query": query[c * B_SH : (c + 1) * B_SH],
        }
        for c in range(N_CORES)
    ]
    res = run_bass_kernel_spmd(
        nc, in_maps, core_ids=list(range(N_CORES)), trace=trace
    )
    full = np.concatenate([r["out"] for r in res.results], axis=0)
    return full, res


def kernel(support: np.ndarray, query: np.ndarray) -> np.ndarray:
    full, _ = run(support, query, trace=False)
    return full


# revision 11
# speedup vs baseline: 2.5929x; 1.1023x over previous
"""DistanceNetwork kernel for 8 TRN2 NeuronCores.

out[b, i] = dot(query[b], support[b, i]) * rsqrt(max(||support[b,i]||^2, EPS))

Sharding: batch dim (2048) split across 8 cores -> 256 batches/core, no
cross-core communication. Per core, batches sit on SBUF partitions (2 blocks
of 128). For each support row i:
  - VectorE: scalar_tensor_tensor (S*1)*Q with accum_out -> dot column
  - ScalarE: activation(Square) with accum_out         -> sqnorm column
Each engine touches every support element exactly once, so both stay under
the per-core HBM floor (~128 MiB / 358 GB/s ~= 374 us). Epilogue per block:
clip -> sqrt -> reciprocal -> multiply -> DMA out.

Raw bass (no TileContext): this container's walrus can't encode multi-wait
instructions that Tile's scheduler emits, so synchronization is explicit —
standalone wait_ge + then_inc, one semaphore per wait. S-tile loads ride the
SP HWDGE ring (FIFO, back-to-back at full SDMA width); query/output DMAs use
the ACT ring so they don't head-of-line block the S stream.

`repeats` re-emits the whole body N times with cumulative semaphore
thresholds — used by bench.py to measure steady-state per-iteration device
time without host dispatch in the loop.
"""

import sys

if "/opt/trn_rl_repo" not in sys.path:
    sys.path.insert(0, "/opt/trn_rl_repo")

from contextlib import ExitStack

import numpy as np

B, CK, D = 2048, 256, 512
N_CORES = 8
B_SH = B // N_CORES   # 256 batches per core
PB = 128              # batches (partitions) per block
NBLK = B_SH // PB     # 2 blocks
G = 16                # support rows per S-tile DMA (G*D*4 = 32 KiB/partition)
TPB_ = CK // G        # 16 S tiles per block
NTILE = NBLK * TPB_   # 32 S tiles per core per repeat
NBUF = 3              # S-tile buffers
EPS = 1e-10

_CACHE = {}


def _build_nc(repeats: int = 1, g: int = G, nbuf: int = NBUF):
    import concourse.bass as bass
    from concourse import mybir

    f32 = mybir.dt.float32
    AF = mybir.ActivationFunctionType
    ALU = mybir.AluOpType

    G_ = g
    TPB_ = CK // G_
    NTILE = NBLK * TPB_
    NBUF_ = nbuf

    nc = bass.Bass(
        trn_type="TRN2",
        target_bir_lowering=False,
        debug=False,
        num_devices=N_CORES,
        # the detector rejects benign same-engine in-order WAW on the
        # scratch tiles (engines execute their stream in order + DRAIN);
        # cross-engine hazards here are explicitly semaphored
        detect_race_conditions=False,
    )
    sup = nc.dram_tensor("support", [B_SH, CK, D], f32, kind="ExternalInput")
    qry = nc.dram_tensor("query", [B_SH, D], f32, kind="ExternalInput")
    out = nc.dram_tensor("out", [B_SH, CK], f32, kind="ExternalOutput")

    sup2 = sup.rearrange("b i d -> b (i d)")  # [B_SH, CK*D], contiguous
    qry2 = qry.ap()
    out2 = out.ap()

    with ExitStack() as ctx:
        e = ctx.enter_context
        s_buf = [e(nc.sbuf_tensor(f"s{n}", [PB, G_ * D], f32)) for n in range(NBUF_)]
        q_buf = [e(nc.sbuf_tensor(f"q{b}", [PB, D], f32)) for b in range(NBLK)]
        dot = [e(nc.sbuf_tensor(f"dot{b}", [PB, CK], f32)) for b in range(NBLK)]
        sq = [e(nc.sbuf_tensor(f"sq{b}", [PB, CK], f32)) for b in range(NBLK)]
        scr_v = e(nc.sbuf_tensor("scr_v", [PB, D], f32))
        scr_a = e(nc.sbuf_tensor("scr_a", [PB, D], f32))
        clip = [e(nc.sbuf_tensor(f"clip{b}", [PB, CK], f32)) for b in range(NBLK)]
        mag = [e(nc.sbuf_tensor(f"mag{b}", [PB, CK], f32)) for b in range(NBLK)]
        inv = [e(nc.sbuf_tensor(f"inv{b}", [PB, CK], f32)) for b in range(NBLK)]
        o_t = [e(nc.sbuf_tensor(f"o{b}", [PB, CK], f32)) for b in range(NBLK)]

        # One completion sem per S buffer slot: a slot's DMAs are strictly
        # serialized by the buffer-reuse waits, so per-slot counts are
        # unambiguous (one shared sem would mix +16s of concurrent DMAs).
        sem_s = [e(nc.semaphore(f"sem_s{n}")) for n in range(NBUF_)]
        sem_q = [e(nc.semaphore(f"sem_q{b}")) for b in range(NBLK)]
        sem_o = e(nc.semaphore("sem_o"))  # output DMA completions (+16 each)
        sem_v = e(nc.semaphore("sem_v"))  # vector: S tiles fully consumed (+1)
        sem_a = e(nc.semaphore("sem_a"))  # scalar: S tiles fully consumed (+1)
        sem_e = e(nc.semaphore("sem_e"))  # vector epilogue steps (+1 each)
        sem_f = e(nc.semaphore("sem_f"))  # scalar epilogue sqrt done (+1)

        # ---- SP stream: big S-tile loads, triple buffered ----
        for r in range(repeats):
            for kk in range(NTILE):
                k = r * NTILE + kk
                if k >= NBUF_:
                    # slot k%NBUF holds tile k-NBUF: both consumers done?
                    nc.sync.wait_ge(sem_v, k - NBUF_ + 1)
                    nc.sync.wait_ge(sem_a, k - NBUF_ + 1)
                b, j = divmod(kk, TPB_)
                row = sup2[b * PB : (b + 1) * PB, j * G_ * D : (j + 1) * G_ * D]
                nc.sync.dma_start(out=s_buf[k % NBUF_].ap(), in_=row).then_inc(
                    sem_s[k % NBUF_], 16
                )
        nc.sync.wait_ge(sem_o, 16 * NBLK * repeats)  # outputs landed

        # ---- ACT stream: q loads, square+accumulate, sqrt, output stores ----
        for r in range(repeats):
            for b in range(NBLK):
                if r > 0:
                    # q_buf[b] may still feed repeat r-1's vector STTs
                    nc.scalar.wait_ge(sem_v, r * NTILE)
                nc.scalar.dma_start(
                    out=q_buf[b].ap(), in_=qry2[b * PB : (b + 1) * PB, :]
                ).then_inc(sem_q[b], 16)
            for b in range(NBLK):
                for j in range(TPB_):
                    k = r * NTILE + b * TPB_ + j
                    nc.scalar.wait_ge(sem_s[k % NBUF_], 16 * (k // NBUF_ + 1))
                    sb = s_buf[k % NBUF_].ap()
                    for g in range(G_):
                        c = j * G_ + g
                        ins = nc.scalar.activation(
                            out=scr_a.ap(),
                            in_=sb[:, g * D : (g + 1) * D],
                            func=AF.Square,
                            accum_out=sq[b].ap()[:, c : c + 1],
                        )
                        if g == G_ - 1:
                            ins.then_inc(sem_a, 1)
                # epilogue: sqrt of clipped norms once vector built clip[b]
                eb = r * NBLK + b
                nc.scalar.wait_ge(sem_e, 2 * eb + 1)
                nc.scalar.activation(mag[b].ap(), clip[b].ap(), AF.Sqrt).then_inc(
                    sem_f, 1
                )
                nc.scalar.wait_ge(sem_e, 2 * eb + 2)
                nc.scalar.dma_start(
                    out=out2[b * PB : (b + 1) * PB, :], in_=o_t[b].ap()
                ).then_inc(sem_o, 16)

        # ---- DVE stream: fused multiply+reduce (dot), epilogue arithmetic ----
        for r in range(repeats):
            for b in range(NBLK):
                nc.vector.wait_ge(sem_q[b], 16 * (r + 1))
                for j in range(TPB_):
                    k = r * NTILE + b * TPB_ + j
                    nc.vector.wait_ge(sem_s[k % NBUF_], 16 * (k // NBUF_ + 1))
                    sb = s_buf[k % NBUF_].ap()
                    for g in range(G_):
                        c = j * G_ + g
                        ins = nc.vector.scalar_tensor_tensor(
                            out=scr_v.ap(),
                            in0=sb[:, g * D : (g + 1) * D],
                            scalar=1.0,
                            in1=q_buf[b].ap(),
                            op0=ALU.mult,
                            op1=ALU.mult,
                            accum_out=dot[b].ap()[:, c : c + 1],
                        )
                        if g == G_ - 1:
                            ins.then_inc(sem_v, 1)
                eb = r * NBLK + b
                nc.vector.wait_ge(sem_a, TPB_ * (NBLK * r + b + 1))
                nc.vector.tensor_scalar_max(clip[b].ap(), sq[b].ap(), EPS).then_inc(
                    sem_e, 1
                )
                nc.vector.wait_ge(sem_f, eb + 1)
                nc.vector.reciprocal(inv[b].ap(), mag[b].ap())
                if r > 0:
                    # o_t[b] may still be read by repeat r-1's output DMA
                    nc.vector.wait_ge(sem_o, 16 * (NBLK * (r - 1) + b + 1))
                nc.vector.tensor_mul(o_t[b].ap(), dot[b].ap(), inv[b].ap()).then_inc(
                    sem_e, 1
                )

    return nc


def run(support: np.ndarray, query: np.ndarray, trace: bool = False):
    """Returns (full_output, BassKernelResults)."""
    from concourse.bass_utils import run_bass_kernel_spmd

    if "nc" not in _CACHE:
        _CACHE["nc"] = _build_nc()
    nc = _CACHE["nc"]

    support = np.ascontiguousarray(np.asarray(support, dtype=np.float32))
    query = np.ascontiguousarray(np.asarray(query, dtype=np.float32))
    in_maps = [
        {
            "support": support[c * B_SH : (c + 1) * B_SH],
            "query": query[c * B_SH : (c + 1) * B_SH],
        }
        for c in range(N_CORES)
    ]
    res = run_bass_kernel_spmd(
        nc, in_maps, core_ids=list(range(N_CORES)), trace=trace
    )
    full = np.concatenate([r["out"] for r in res.results], axis=0)
    return full, res


def kernel(support: np.ndarray, query: np.ndarray) -> np.ndarray:
    full, _ = run(support, query, trace=False)
    return full


# revision 13
# speedup vs baseline: 3.0255x; 1.1668x over previous
"""DistanceNetwork kernel for 8 TRN2 NeuronCores.

out[b, i] = dot(query[b], support[b, i]) * rsqrt(max(||support[b,i]||^2, EPS))

Sharding: batch dim (2048) split across 8 cores -> 256 batches/core, no
cross-core communication. Per core, batches sit on SBUF partitions (2 blocks
of 128). For each support row i:
  - VectorE: scalar_tensor_tensor (S*1)*Q with accum_out -> dot column
  - ScalarE: activation(Square) with accum_out         -> sqnorm column
Each engine touches every support element exactly once, so both stay under
the per-core HBM floor (~128 MiB / 358 GB/s ~= 374 us). Epilogue per block:
clip -> sqrt -> reciprocal -> multiply -> DMA out.

Raw bass (no TileContext): this container's walrus can't encode multi-wait
instructions that Tile's scheduler emits, so synchronization is explicit —
standalone wait_ge + then_inc, one semaphore per wait. S-tile loads ride the
SP HWDGE ring (FIFO, back-to-back at full SDMA width); query/output DMAs use
the ACT ring so they don't head-of-line block the S stream.

`repeats` re-emits the whole body N times with cumulative semaphore
thresholds — used by bench.py to measure steady-state per-iteration device
time without host dispatch in the loop.
"""

import sys

if "/opt/trn_rl_repo" not in sys.path:
    sys.path.insert(0, "/opt/trn_rl_repo")

from contextlib import ExitStack

import numpy as np

B, CK, D = 2048, 256, 512
N_CORES = 8
B_SH = B // N_CORES   # 256 batches per core
PB = 128              # batches (partitions) per block
NBLK = B_SH // PB     # 2 blocks
G = 16                # support rows per S-tile DMA (G*D*4 = 32 KiB/partition)
TPB_ = CK // G        # 16 S tiles per block
NTILE = NBLK * TPB_   # 32 S tiles per core per repeat
NBUF = 3              # S-tile buffers
VSQ = 6               # squares per tile computed on VectorE (rest on ScalarE)
EPS = 1e-10

_CACHE = {}


def _build_nc(repeats: int = 1, g: int = G, nbuf: int = NBUF, vsq: int = VSQ):
    import concourse.bass as bass
    from concourse import mybir

    f32 = mybir.dt.float32
    AF = mybir.ActivationFunctionType
    ALU = mybir.AluOpType

    G_ = g
    TPB_ = CK // G_
    NTILE = NBLK * TPB_
    NBUF_ = nbuf

    nc = bass.Bass(
        trn_type="TRN2",
        target_bir_lowering=False,
        debug=False,
        num_devices=N_CORES,
        # the detector rejects benign same-engine in-order WAW on the
        # scratch tiles (engines execute their stream in order + DRAIN);
        # cross-engine hazards here are explicitly semaphored
        detect_race_conditions=False,
    )
    sup = nc.dram_tensor("support", [B_SH, CK, D], f32, kind="ExternalInput")
    qry = nc.dram_tensor("query", [B_SH, D], f32, kind="ExternalInput")
    out = nc.dram_tensor("out", [B_SH, CK], f32, kind="ExternalOutput")

    sup2 = sup.rearrange("b i d -> b (i d)")  # [B_SH, CK*D], contiguous
    qry2 = qry.ap()
    out2 = out.ap()

    # per-partition EPS bias for the epilogue sqrt(sq + EPS)
    eps_t = nc.alloc_sbuf_tensor("const_eps", [PB, 1], f32)
    nc.gpsimd.memset(eps_t.ap(), EPS)
    nc.all_engine_barrier()

    with ExitStack() as ctx:
        e = ctx.enter_context
        s_buf = [e(nc.sbuf_tensor(f"s{n}", [PB, G_ * D], f32)) for n in range(NBUF_)]
        q_buf = [e(nc.sbuf_tensor(f"q{b}", [PB, D], f32)) for b in range(NBLK)]
        dot = [e(nc.sbuf_tensor(f"dot{b}", [PB, CK], f32)) for b in range(NBLK)]
        sq = [e(nc.sbuf_tensor(f"sq{b}", [PB, CK], f32)) for b in range(NBLK)]
        scr_v = e(nc.sbuf_tensor("scr_v", [PB, D], f32))
        scr_a = e(nc.sbuf_tensor("scr_a", [PB, D], f32))
        mag = [e(nc.sbuf_tensor(f"mag{b}", [PB, CK], f32)) for b in range(NBLK)]
        inv = [e(nc.sbuf_tensor(f"inv{b}", [PB, CK], f32)) for b in range(NBLK)]
        o_t = [e(nc.sbuf_tensor(f"o{b}", [PB, CK], f32)) for b in range(NBLK)]

        # One completion sem per S buffer slot: a slot's DMAs are strictly
        # serialized by the buffer-reuse waits, so per-slot counts are
        # unambiguous (one shared sem would mix +16s of concurrent DMAs).
        sem_s = [e(nc.semaphore(f"sem_s{n}")) for n in range(NBUF_)]
        sem_q = [e(nc.semaphore(f"sem_q{b}")) for b in range(NBLK)]
        sem_o = e(nc.semaphore("sem_o"))  # output DMA completions (+16 each)
        sem_v = e(nc.semaphore("sem_v"))  # vector: S tiles fully consumed (+1)
        sem_a = e(nc.semaphore("sem_a"))  # scalar: S tiles fully consumed (+1)
        sem_e = e(nc.semaphore("sem_e"))  # vector epilogue steps (+1 each)
        sem_f = e(nc.semaphore("sem_f"))  # scalar epilogue sqrt done (+1)

        # ---- SP stream: big S-tile loads, triple buffered ----
        for r in range(repeats):
            for kk in range(NTILE):
                k = r * NTILE + kk
                if k >= NBUF_:
                    # slot k%NBUF holds tile k-NBUF: both consumers done?
                    nc.sync.wait_ge(sem_v, k - NBUF_ + 1)
                    nc.sync.wait_ge(sem_a, k - NBUF_ + 1)
                b, j = divmod(kk, TPB_)
                row = sup2[b * PB : (b + 1) * PB, j * G_ * D : (j + 1) * G_ * D]
                nc.sync.dma_start(out=s_buf[k % NBUF_].ap(), in_=row).then_inc(
                    sem_s[k % NBUF_], 16
                )
        nc.sync.wait_ge(sem_o, 16 * NBLK * repeats)  # outputs landed

        # ---- ACT stream: q loads, its share of squares, sqrt, stores ----
        for r in range(repeats):
            for b in range(NBLK):
                if r > 0:
                    # q_buf[b] may still feed repeat r-1's vector STTs
                    nc.scalar.wait_ge(sem_v, r * NTILE)
                nc.scalar.dma_start(
                    out=q_buf[b].ap(), in_=qry2[b * PB : (b + 1) * PB, :]
                ).then_inc(sem_q[b], 16)
            for b in range(NBLK):
                for j in range(TPB_):
                    k = r * NTILE + b * TPB_ + j
                    nc.scalar.wait_ge(sem_s[k % NBUF_], 16 * (k // NBUF_ + 1))
                    sb = s_buf[k % NBUF_].ap()
                    for g in range(vsq, G_):
                        c = j * G_ + g
                        ins = nc.scalar.activation(
                            out=scr_a.ap(),
                            in_=sb[:, g * D : (g + 1) * D],
                            func=AF.Square,
                            accum_out=sq[b].ap()[:, c : c + 1],
                        )
                        if g == G_ - 1:
                            ins.then_inc(sem_a, 1)
            # epilogue after both blocks: adjacent sqrts share one ACT
            # table-set switch; mag = sqrt(sq + EPS) (== sqrt(max(sq, EPS))
            # to fp32 precision for any non-degenerate support row)
            for b in range(NBLK):
                nc.scalar.wait_ge(sem_v, r * NTILE + TPB_ * (b + 1))
                nc.scalar.activation(
                    mag[b].ap(), sq[b].ap(), AF.Sqrt, bias=eps_t.ap()
                ).then_inc(sem_f, 1)
            for b in range(NBLK):
                nc.scalar.wait_ge(sem_e, NBLK * r + b + 1)
                nc.scalar.dma_start(
                    out=out2[b * PB : (b + 1) * PB, :], in_=o_t[b].ap()
                ).then_inc(sem_o, 16)

        # ---- DVE stream: all dots + its share of squares, epilogue ----
        for r in range(repeats):
            for b in range(NBLK):
                nc.vector.wait_ge(sem_q[b], 16 * (r + 1))
                for j in range(TPB_):
                    k = r * NTILE + b * TPB_ + j
                    nc.vector.wait_ge(sem_s[k % NBUF_], 16 * (k // NBUF_ + 1))
                    sb = s_buf[k % NBUF_].ap()
                    for g in range(G_):
                        c = j * G_ + g
                        nc.vector.scalar_tensor_tensor(
                            out=scr_v.ap(),
                            in0=sb[:, g * D : (g + 1) * D],
                            scalar=1.0,
                            in1=q_buf[b].ap(),
                            op0=ALU.mult,
                            op1=ALU.mult,
                            accum_out=dot[b].ap()[:, c : c + 1],
                        )
                    for g in range(vsq):
                        c = j * G_ + g
                        sl = sb[:, g * D : (g + 1) * D]
                        ins = nc.vector.scalar_tensor_tensor(
                            out=scr_v.ap(),
                            in0=sl,
                            scalar=1.0,
                            in1=sl,
                            op0=ALU.mult,
                            op1=ALU.mult,
                            accum_out=sq[b].ap()[:, c : c + 1],
                        )
                        if g == vsq - 1:
                            ins.then_inc(sem_v, 1)
            for b in range(NBLK):
                eb = r * NBLK + b
                nc.vector.wait_ge(sem_f, eb + 1)
                nc.vector.reciprocal(inv[b].ap(), mag[b].ap())
                if r > 0:
                    # o_t[b] may still be read by repeat r-1's output DMA
                    nc.vector.wait_ge(sem_o, 16 * (NBLK * (r - 1) + b + 1))
                nc.vector.tensor_mul(o_t[b].ap(), dot[b].ap(), inv[b].ap()).then_inc(
                    sem_e, 1
                )

    return nc


def run(support: np.ndarray, query: np.ndarray, trace: bool = False):
    """Returns (full_output, BassKernelResults)."""
    from concourse.bass_utils import run_bass_kernel_spmd

    if "nc" not in _CACHE:
        _CACHE["nc"] = _build_nc()
    nc = _CACHE["nc"]

    support = np.ascontiguousarray(np.asarray(support, dtype=np.float32))
    query = np.ascontiguousarray(np.asarray(query, dtype=np.float32))
    in_maps = [
        {
            "support": support[c * B_SH : (c + 1) * B_SH],
            "query": query[c * B_SH : (c + 1) * B_SH],
        }
        for c in range(N_CORES)
    ]
    res = run_bass_kernel_spmd(
        nc, in_maps, core_ids=list(range(N_CORES)), trace=trace
    )
    full = np.concatenate([r["out"] for r in res.results], axis=0)
    return full, res


def kernel(support: np.ndarray, query: np.ndarray) -> np.ndarray:
    full, _ = run(support, query, trace=False)
    return full
